# revision 25
# baseline (speedup 1.0000x reference)
"""Trainium2 Bass kernel for a SqueezeNet Fire module.

    x [32, 512, 56, 56] fp32
    s  = relu(squeeze_w @ x + squeeze_b)          # 1x1, 512 -> 64
    e1 = relu(expand1x1_w @ s + expand1x1_b)      # 1x1, 64 -> 256
    e3 = relu(conv3x3(s, expand3x3_w) + b)        # 3x3 pad 1, 64 -> 256
    out = concat([e1, e3], channel)               # [32, 512, 56, 56] fp32

Sharding: data-parallel over batch, 4 images per NeuronCore x 8 cores.

Per-core plan (per image, spatial flattened to 56x56=3136, chunked 7x448):
  - squeeze: 4 accumulating K=128 matmuls, stationary duplicated along M
    (64 -> 128) so one relu+bias eviction fills both halves of a padded
    S buffer SS [128, 58, 58] (partitions 0-63 = copy A, 64-127 = copy B).
  - expand1x1 / expand3x3: K=64 matmuls issued as h0/h1 pairs (concurrent
    PE row-group tiles); expand3x3 = 9 shifted-tap matmuls accumulating in
    PSUM over shifted windows of SS.
  - x prefetched 4 groups ahead on the gpsimd SWDGE queue; outputs drain
    on the sync HWDGE queue; fill ramps at squeeze-lead 2 with per-chunk
    interleave, then steady state emits squeeze in 2-chunk batches
    ([sq sq | e e]) so the PE pays the ~98ns 64<->128-row stationary
    reconfiguration twice per two chunks instead of four times.

Schedule notes (measured on HW, 12 runs, 101-126us):
  - The DVFS governor only steps the tensor engine to the full 2.4GHz
    p-state (352ns per 448-col matmul) when the early instruction stream
    has idle windows; the fill's natural x-wait stalls (~0.5-2.5us around
    13-17us) provide them. Fully gap-free fills (PE warm-up dummies,
    aggressive multi-queue fill) plateaued at ~1.74GHz (422ns/matmul),
    costing ~15us despite perfect overlap. Once ramped, the governor
    sustains 2.4GHz through the packed batched stream. Do not remove the
    fill stalls. (The granted p-state also varies with ambient chip load;
    this config measured 101.0/100.8us on consecutive runs.)
  - Staging any x input on the sync HWDGE queue delays the 12.85MB output
    drain queued behind it (+15us tail): x belongs on gpsimd SWDGE only.
    The scalar HWDGE queue is ~3x slower than sync/gpsimd (~70-120B/ns).
  - The tile framework recycles DMA completion semaphores with cumulative
    thresholds: issuing loads far ahead of consumption creates false
    waits on much-later DMAs. Keep the 4-group prefetch pacing.

I/O is bf16 both ways (cast on host); matmul operands bf16 with fp32
PSUM accumulation. Relative error vs fp32 reference: 4.4e-3.
"""

import sys

if "/opt/trn_rl_repo" not in sys.path:
    sys.path.insert(0, "/opt/trn_rl_repo")

import ml_dtypes
import numpy as np

import concourse.bass as bass
import concourse.tile as tile
from concourse import bacc, mybir

F32 = mybir.dt.float32
F32R = mybir.dt.float32r
BF16 = mybir.dt.bfloat16
RELU = mybir.ActivationFunctionType.Relu

N_CORES = 8
N_TOTAL, C_IN, H, W = 32, 512, 56, 56
N_IMG = N_TOTAL // N_CORES          # images per core
C_SQ, C_E = 64, 256                 # squeeze / expand channels
HW = H * W                          # 3136
ROWS_PER_CHUNK = 8
N_CHUNK = H // ROWS_PER_CHUNK       # 7 chunks of 8 rows
CHUNK = ROWS_PER_CHUNK * W          # 448 spatial positions per chunk
HP, WP = H + 2, W + 2               # padded S frame 58x58
K_TILES = C_IN // 128               # 4

IN_BF16 = True
EXP_BF16 = True
OUT_BF16 = True


def _build(in_bf16, exp_bf16, out_bf16):
    xdt = BF16 if in_bf16 else F32R
    edt = BF16 if exp_bf16 else F32R
    odt = BF16 if out_bf16 else F32
    nc = bacc.Bacc("TRN2", target_bir_lowering=False, debug=False,
                   num_devices=N_CORES)
    x_d = nc.dram_tensor("x", [N_IMG, 128, K_TILES, HW], xdt,
                         kind="ExternalInput").ap()
    wsq_d = nc.dram_tensor("wsq", [128, K_TILES, 128], xdt,
                           kind="ExternalInput").ap()
    w1_d = nc.dram_tensor("w1", [128, 128], edt, kind="ExternalInput").ap()
    w3_d = nc.dram_tensor("w3", [128, 9, 128], edt, kind="ExternalInput").ap()
    bsq_d = nc.dram_tensor("bsq", [128, 1], F32, kind="ExternalInput").ap()
    b1_d = nc.dram_tensor("b1", [128, 2], F32, kind="ExternalInput").ap()
    b3_d = nc.dram_tensor("b3", [128, 2], F32, kind="ExternalInput").ap()
    out_d = nc.dram_tensor("out", [N_IMG, 2 * C_E, HW], odt,
                           kind="ExternalOutput").ap()

    with tile.TileContext(nc) as tc:
        with (
            tc.tile_pool(name="wpool", bufs=1) as wpool,
            tc.tile_pool(name="xpool", bufs=6) as xpool,
            tc.tile_pool(name="sspool", bufs=2) as sspool,
            tc.tile_pool(name="opool", bufs=4) as opool,
            tc.tile_pool(name="psum", bufs=1, space="PSUM") as psum,
        ):
            wsq_t = wpool.tile([128, K_TILES, 128], xdt)
            w1_t = wpool.tile([128, 128], edt)
            w3_t = wpool.tile([128, 9, 128], edt)
            bsq_t = wpool.tile([128, 1], F32)
            b1_t = wpool.tile([128, 2], F32)
            b3_t = wpool.tile([128, 2], F32)
            nc.sync.dma_start(wsq_t[:], wsq_d[:])
            nc.sync.dma_start(w1_t[:], w1_d[:])
            nc.sync.dma_start(w3_t[:], w3_d[:])
            nc.sync.dma_start(bsq_t[:], bsq_d[:])
            nc.sync.dma_start(b1_t[:], b1_d[:])
            nc.sync.dma_start(b3_t[:], b3_d[:])

            warm = wpool.tile([1, 1], F32)
            nc.vector.memset(warm[:], 0.0)
            nc.scalar.activation(warm[:], warm[:], RELU)

            x_tiles = {}
            ss_tiles = {}
            out_stage = [None] * 4

            def load_group(n, g, eng=None):
                w = min(2 * CHUNK, HW - 2 * g * CHUNK)
                t = xpool.tile([128, K_TILES, w], xdt, tag="xc",
                               name=f"xc_{n}_{g}")
                if n == 0:
                    for c in range(0, w, CHUNK):
                        nc.gpsimd.dma_start(
                            t[:, :, c : c + CHUNK],
                            x_d[n, :, :,
                                2 * g * CHUNK + c : 2 * g * CHUNK + c + CHUNK],
                        )
                else:
                    nc.gpsimd.dma_start(
                        t[:], x_d[n, :, :, 2 * g * CHUNK : 2 * g * CHUNK + w]
                    )
                x_tiles[(n, g)] = t

            def setup_image(n):
                ss = sspool.tile([128, HP, WP], edt, tag="ss")
                mdt = BF16 if exp_bf16 else F32
                nc.vector.memset(ss[:, 0, :].bitcast(mdt), 0.0)
                nc.vector.memset(ss[:, HP - 1, :].bitcast(mdt), 0.0)
                nc.vector.memset(ss[:, 1 : HP - 1, 0].bitcast(mdt), 0.0)
                nc.vector.memset(ss[:, 1 : HP - 1, WP - 1].bitcast(mdt), 0.0)
                ss_tiles[n] = ss

            def squeeze_chunk(n, j):
                if n not in ss_tiles:
                    setup_image(n)
                ps = psum.tile([128, ROWS_PER_CHUNK, W], F32, tag="sq", bufs=2,
                               name=f"sq_{n}_{j}")
                xt = x_tiles[(n, j // 2)]
                c0 = (j % 2) * CHUNK
                for k in range(K_TILES):
                    nc.tensor.matmul(
                        ps[:],
                        wsq_t[:, k, :],
                        xt[:, k, c0 : c0 + CHUNK],
                        start=(k == 0),
                        stop=(k == K_TILES - 1),
                    )
                y0 = j * ROWS_PER_CHUNK
                dst = ss_tiles[n][:, 1 + y0 : 1 + y0 + ROWS_PER_CHUNK, 1 : 1 + W]
                if j % 2 == 0:
                    nc.scalar.activation(dst, ps[:], RELU, bias=bsq_t[:])
                else:
                    nc.vector.tensor_scalar(
                        dst, ps[:], bsq_t[:], 0.0,
                        op0=mybir.AluOpType.add, op1=mybir.AluOpType.max,
                    )

            e_state = {}

            def expand_chunk_mm(n, j, taps):
                ss = ss_tiles[n]
                y0 = j * ROWS_PER_CHUNK
                if taps[0] == 0:
                    p1 = [psum.tile([128, CHUNK], F32, tag=f"e1h{h}", bufs=1,
                                    name=f"p1h{h}_{n}_{j}")
                          for h in range(2)]
                    p3 = [psum.tile([128, CHUNK], F32, tag=f"e3h{h}", bufs=2,
                                    name=f"p3h{h}_{n}_{j}")
                          for h in range(2)]
                    e_state[(n, j)] = (p1, p3)
                    for h in range(2):
                        nc.tensor.matmul(
                            p1[h][:],
                            w1_t[64 * h : 64 * h + 64, :],
                            ss[64 * h : 64 * h + 64,
                               1 + y0 : 1 + y0 + ROWS_PER_CHUNK, 1 : 1 + W],
                            start=True,
                            stop=True,
                        )
                p1, p3 = e_state[(n, j)]
                for t in taps:
                    dy, dx = t // 3, t % 3
                    for h in range(2):
                        nc.tensor.matmul(
                            p3[h][:],
                            w3_t[64 * h : 64 * h + 64, t, :],
                            ss[64 * h : 64 * h + 64,
                               y0 + dy : y0 + dy + ROWS_PER_CHUNK,
                               dx : dx + W],
                            start=(t == 0),
                            stop=(t == 8),
                        )
            def expand_chunk_evict(n, j):
                p1, p3 = e_state.pop((n, j))
                last = n == N_IMG - 1 and j == N_CHUNK - 1
                g, half = j // 2, j % 2
                gw = 1 if j == N_CHUNK - 1 else 2
                if half == 0:
                    for role in range(4):
                        out_stage[role] = opool.tile(
                            [128, gw, CHUNK], odt, tag=f"o{role}",
                            name=f"o{role}_{n}_{g}")
                for h in range(2):
                    nc.vector.tensor_scalar(
                        out_stage[h][:, half, :], p1[h][:],
                        b1_t[:, h : h + 1], 0.0,
                        op0=mybir.AluOpType.add, op1=mybir.AluOpType.max,
                    )
                nc.scalar.activation(out_stage[2][:, half, :],
                                     p3[0][:], RELU, bias=b3_t[:, 0:1])
                if last:
                    # final chunk only: second e3 half evicts on DVE so the
                    # two evictions run concurrently instead of queuing on
                    # ACT — shortens the drain, touches nothing early
                    nc.vector.tensor_scalar(
                        out_stage[3][:, half, :], p3[1][:],
                        b3_t[:, 1:2], 0.0,
                        op0=mybir.AluOpType.add, op1=mybir.AluOpType.max,
                    )
                else:
                    nc.scalar.activation(out_stage[3][:, half, :],
                                         p3[1][:], RELU, bias=b3_t[:, 1:2])
                if half + 1 == gw:
                    # final group only: fan the 4 drain DMAs across three
                    # queues instead of serializing ~2us on sync
                    engines = [nc.sync] * 4
                    if last:
                        engines = [nc.sync, nc.sync, nc.scalar, nc.gpsimd]
                    for role in range(4):
                        ch0 = 128 * role
                        engines[role].dma_start(
                            out_d[n, ch0 : ch0 + 128,
                                  2 * g * CHUNK : (2 * g + gw) * CHUNK],
                            out_stage[role][:],
                        )

            PREFETCH = 4
            chunks = [(n, j) for n in range(N_IMG) for j in range(N_CHUNK)]
            groups = []
            for n in range(N_IMG):
                for g in range((N_CHUNK + 1) // 2):
                    groups.append((n, g))
            for gi in range(min(PREFETCH, len(groups))):
                load_group(*groups[gi])
            st = {"next_load": PREFETCH}
            ALL = list(range(9))

            def sq(i):
                n2, j2 = chunks[i]
                if (n2, j2 // 2) not in x_tiles and st["next_load"] < len(groups):
                    load_group(*groups[st["next_load"]])
                    st["next_load"] += 1
                squeeze_chunk(n2, j2)

            # Fill ramp at squeeze-lead 2 with per-chunk interleave — its
            # natural x-wait stalls (~2.5us around 13-17us) are what lets
            # the DVFS governor step the PE to the full 2.4GHz p-state.
            # Steady state then emits squeeze in 2-chunk batches so the PE
            # pays the ~98ns 64<->128-row stationary reconfiguration twice
            # per two chunks instead of four times; once ramped, the
            # governor sustains 2.4GHz through the packed batch stream.
            sq(0)
            sq(1)
            sq(2)
            expand_chunk_mm(*chunks[0], ALL)
            expand_chunk_evict(*chunks[0])
            sq(3)
            expand_chunk_mm(*chunks[1], ALL)
            expand_chunk_evict(*chunks[1])
            for i in range(2, len(chunks), 2):
                if i + 2 < len(chunks):
                    sq(i + 2)
                if i + 3 < len(chunks):
                    sq(i + 3)
                expand_chunk_mm(*chunks[i], ALL)
                expand_chunk_evict(*chunks[i])
                expand_chunk_mm(*chunks[i + 1], ALL)
                expand_chunk_evict(*chunks[i + 1])

    nc.compile()
    return nc


_NC_CACHE = {}


def _get_nc(in_bf16=IN_BF16, exp_bf16=EXP_BF16, out_bf16=OUT_BF16):
    key = (in_bf16, exp_bf16, out_bf16)
    if key not in _NC_CACHE:
        _NC_CACHE[key] = _build(in_bf16, exp_bf16, out_bf16)
    return _NC_CACHE[key]


def _pack_inputs(x, squeeze_w, squeeze_b, expand1x1_w, expand1x1_b,
                 expand3x3_w, expand3x3_b, in_bf16=IN_BF16, exp_bf16=EXP_BF16):
    f = np.float32
    xdt = ml_dtypes.bfloat16 if in_bf16 else f
    edt = ml_dtypes.bfloat16 if exp_bf16 else f
    wsq = np.ascontiguousarray(
        np.tile(squeeze_w, (2, 1))
        .T.reshape(K_TILES, 128, 128)
        .transpose(1, 0, 2)
    ).astype(xdt)
    w1 = np.concatenate(
        [expand1x1_w[:128].T, expand1x1_w[128:].T], axis=0
    ).astype(edt)
    w3e = expand3x3_w.reshape(2, 128, C_SQ, 9)
    w3 = np.ascontiguousarray(w3e.transpose(0, 2, 3, 1)).reshape(128, 9, 128)
    w3 = w3.astype(edt)
    bsq = np.tile(squeeze_b, 2).reshape(128, 1).astype(f)
    b1 = np.ascontiguousarray(expand1x1_b.reshape(2, 128).T).astype(f)
    b3 = np.ascontiguousarray(expand3x3_b.reshape(2, 128).T).astype(f)
    xs = np.ascontiguousarray(
        x.reshape(N_CORES, N_IMG, K_TILES, 128, HW).transpose(0, 1, 3, 2, 4)
    ).astype(xdt)
    return xs, {"wsq": wsq, "w1": w1, "w3": w3, "bsq": bsq, "b1": b1, "b3": b3}


def _run(inputs, trace=False, in_bf16=IN_BF16, exp_bf16=EXP_BF16,
         out_bf16=OUT_BF16):
    from concourse import bass_utils

    nc = _get_nc(in_bf16, exp_bf16, out_bf16)
    xs, weights = _pack_inputs(**inputs, in_bf16=in_bf16, exp_bf16=exp_bf16)
    in_maps = [{"x": xs[c], **weights} for c in range(N_CORES)]
    res = bass_utils.run_bass_kernel_spmd(
        nc, in_maps, core_ids=list(range(N_CORES)), trace=trace
    )
    out = np.concatenate([res.results[c]["out"] for c in range(N_CORES)], axis=0)
    return out.reshape(N_TOTAL, 2 * C_E, H, W).astype(np.float32), res


def kernel(**inputs) -> np.ndarray:
    inputs = {k: np.asarray(v, dtype=np.float32) for k, v in inputs.items()}
    out, _ = _run(inputs, trace=False)
    return out


# revision 26
# speedup vs baseline: 1.0090x; 1.0090x over previous
"""Trainium2 Bass kernel for a SqueezeNet Fire module.

    x [32, 512, 56, 56] fp32
    s  = relu(squeeze_w @ x + squeeze_b)          # 1x1, 512 -> 64
    e1 = relu(expand1x1_w @ s + expand1x1_b)      # 1x1, 64 -> 256
    e3 = relu(conv3x3(s, expand3x3_w) + b)        # 3x3 pad 1, 64 -> 256
    out = concat([e1, e3], channel)               # [32, 512, 56, 56] fp32

Sharding: data-parallel over batch, 4 images per NeuronCore x 8 cores.

Per-core plan (per image, spatial flattened to 56x56=3136, chunked 7x448):
  - squeeze: 4 accumulating K=128 matmuls, stationary duplicated along M
    (64 -> 128) so one relu+bias eviction fills both halves of a padded
    S buffer SS [128, 58, 58] (partitions 0-63 = copy A, 64-127 = copy B).
  - expand1x1 / expand3x3: K=64 matmuls issued as h0/h1 pairs (concurrent
    PE row-group tiles); expand3x3 = 9 shifted-tap matmuls accumulating in
    PSUM over shifted windows of SS.
  - x prefetched 4 groups ahead on the gpsimd SWDGE queue; outputs drain
    on the sync HWDGE queue; fill ramps at squeeze-lead 2 with per-chunk
    interleave, then steady state emits squeeze in 2-chunk batches
    ([sq sq | e e]) so the PE pays the ~98ns 64<->128-row stationary
    reconfiguration twice per two chunks instead of four times.

Schedule notes (measured on HW, 12 runs, 101-126us):
  - The DVFS governor only steps the tensor engine to the full 2.4GHz
    p-state (352ns per 448-col matmul) when the early instruction stream
    has idle windows; the fill's natural x-wait stalls (~0.5-2.5us around
    13-17us) provide them. Fully gap-free fills (PE warm-up dummies,
    aggressive multi-queue fill) plateaued at ~1.74GHz (422ns/matmul),
    costing ~15us despite perfect overlap. Once ramped, the governor
    sustains 2.4GHz through the packed batched stream. Do not remove the
    fill stalls. (The granted p-state also varies with ambient chip load;
    this config measured 101.0/100.8us on consecutive runs.)
  - Staging any x input on the sync HWDGE queue delays the 12.85MB output
    drain queued behind it (+15us tail): x belongs on gpsimd SWDGE only.
    The scalar HWDGE queue is ~3x slower than sync/gpsimd (~70-120B/ns).
  - The tile framework recycles DMA completion semaphores with cumulative
    thresholds: issuing loads far ahead of consumption creates false
    waits on much-later DMAs. Keep the 4-group prefetch pacing.

I/O is bf16 both ways (cast on host); matmul operands bf16 with fp32
PSUM accumulation. Relative error vs fp32 reference: 4.4e-3.
"""

import sys

if "/opt/trn_rl_repo" not in sys.path:
    sys.path.insert(0, "/opt/trn_rl_repo")

import ml_dtypes
import numpy as np

import concourse.bass as bass
import concourse.tile as tile
from concourse import bacc, mybir

F32 = mybir.dt.float32
F32R = mybir.dt.float32r
BF16 = mybir.dt.bfloat16
RELU = mybir.ActivationFunctionType.Relu

N_CORES = 8
N_TOTAL, C_IN, H, W = 32, 512, 56, 56
N_IMG = N_TOTAL // N_CORES          # images per core
C_SQ, C_E = 64, 256                 # squeeze / expand channels
HW = H * W                          # 3136
ROWS_PER_CHUNK = 8
N_CHUNK = H // ROWS_PER_CHUNK       # 7 chunks of 8 rows
CHUNK = ROWS_PER_CHUNK * W          # 448 spatial positions per chunk
HP, WP = H + 2, W + 2               # padded S frame 58x58
K_TILES = C_IN // 128               # 4

IN_BF16 = True
EXP_BF16 = True
OUT_BF16 = True


def _build(in_bf16, exp_bf16, out_bf16):
    xdt = BF16 if in_bf16 else F32R
    edt = BF16 if exp_bf16 else F32R
    odt = BF16 if out_bf16 else F32
    nc = bacc.Bacc("TRN2", target_bir_lowering=False, debug=False,
                   num_devices=N_CORES)
    x_d = nc.dram_tensor("x", [N_IMG, 128, K_TILES, HW], xdt,
                         kind="ExternalInput").ap()
    wsq_d = nc.dram_tensor("wsq", [128, K_TILES, 128], xdt,
                           kind="ExternalInput").ap()
    w1_d = nc.dram_tensor("w1", [128, 128], edt, kind="ExternalInput").ap()
    w3_d = nc.dram_tensor("w3", [128, 9, 128], edt, kind="ExternalInput").ap()
    bsq_d = nc.dram_tensor("bsq", [128, 1], F32, kind="ExternalInput").ap()
    b1_d = nc.dram_tensor("b1", [128, 2], F32, kind="ExternalInput").ap()
    b3_d = nc.dram_tensor("b3", [128, 2], F32, kind="ExternalInput").ap()
    out_d = nc.dram_tensor("out", [N_IMG, 2 * C_E, HW], odt,
                           kind="ExternalOutput").ap()

    with tile.TileContext(nc) as tc:
        with (
            tc.tile_pool(name="wpool", bufs=1) as wpool,
            tc.tile_pool(name="xpool", bufs=6) as xpool,
            tc.tile_pool(name="sspool", bufs=2) as sspool,
            tc.tile_pool(name="opool", bufs=4) as opool,
            tc.tile_pool(name="psum", bufs=1, space="PSUM") as psum,
        ):
            wsq_t = wpool.tile([128, K_TILES, 128], xdt)
            w1_t = wpool.tile([128, 128], edt)
            w3_t = wpool.tile([128, 9, 128], edt)
            bsq_t = wpool.tile([128, 1], F32)
            b1_t = wpool.tile([128, 2], F32)
            b3_t = wpool.tile([128, 2], F32)
            nc.sync.dma_start(wsq_t[:], wsq_d[:])
            nc.sync.dma_start(w1_t[:], w1_d[:])
            nc.sync.dma_start(w3_t[:], w3_d[:])
            nc.sync.dma_start(bsq_t[:], bsq_d[:])
            nc.sync.dma_start(b1_t[:], b1_d[:])
            nc.sync.dma_start(b3_t[:], b3_d[:])

            warm = wpool.tile([1, 1], F32)
            nc.vector.memset(warm[:], 0.0)
            nc.scalar.activation(warm[:], warm[:], RELU)

            x_tiles = {}
            ss_tiles = {}
            out_stage = [None] * 4

            def load_group(n, g, eng=None):
                w = min(2 * CHUNK, HW - 2 * g * CHUNK)
                t = xpool.tile([128, K_TILES, w], xdt, tag="xc",
                               name=f"xc_{n}_{g}")
                if n == 0:
                    for c in range(0, w, CHUNK):
                        s0 = 2 * g * CHUNK + c
                        if g == 0 and c == 0:
                            # very first chunk: two k-half DMAs, so the
                            # first squeeze matmuls (k0,k1) start ~1us
                            # before the k2,k3 half has landed
                            for k0 in (0, 2):
                                nc.gpsimd.dma_start(
                                    t[:, k0 : k0 + 2, c : c + CHUNK],
                                    x_d[n, :, k0 : k0 + 2, s0 : s0 + CHUNK],
                                )
                        else:
                            nc.gpsimd.dma_start(
                                t[:, :, c : c + CHUNK],
                                x_d[n, :, :, s0 : s0 + CHUNK],
                            )
                else:
                    nc.gpsimd.dma_start(
                        t[:], x_d[n, :, :, 2 * g * CHUNK : 2 * g * CHUNK + w]
                    )
                x_tiles[(n, g)] = t

            def setup_image(n):
                ss = sspool.tile([128, HP, WP], edt, tag="ss")
                mdt = BF16 if exp_bf16 else F32
                nc.vector.memset(ss[:, 0, :].bitcast(mdt), 0.0)
                nc.vector.memset(ss[:, HP - 1, :].bitcast(mdt), 0.0)
                nc.vector.memset(ss[:, 1 : HP - 1, 0].bitcast(mdt), 0.0)
                nc.vector.memset(ss[:, 1 : HP - 1, WP - 1].bitcast(mdt), 0.0)
                ss_tiles[n] = ss

            def squeeze_chunk(n, j):
                if n not in ss_tiles:
                    setup_image(n)
                ps = psum.tile([128, ROWS_PER_CHUNK, W], F32, tag="sq", bufs=2,
                               name=f"sq_{n}_{j}")
                xt = x_tiles[(n, j // 2)]
                c0 = (j % 2) * CHUNK
                for k in range(K_TILES):
                    nc.tensor.matmul(
                        ps[:],
                        wsq_t[:, k, :],
                        xt[:, k, c0 : c0 + CHUNK],
                        start=(k == 0),
                        stop=(k == K_TILES - 1),
                    )
                y0 = j * ROWS_PER_CHUNK
                dst = ss_tiles[n][:, 1 + y0 : 1 + y0 + ROWS_PER_CHUNK, 1 : 1 + W]
                if j % 2 == 0:
                    nc.scalar.activation(dst, ps[:], RELU, bias=bsq_t[:])
                else:
                    nc.vector.tensor_scalar(
                        dst, ps[:], bsq_t[:], 0.0,
                        op0=mybir.AluOpType.add, op1=mybir.AluOpType.max,
                    )

            e_state = {}

            def expand_chunk_mm(n, j, taps):
                ss = ss_tiles[n]
                y0 = j * ROWS_PER_CHUNK
                if taps[0] == 0:
                    p1 = [psum.tile([128, CHUNK], F32, tag=f"e1h{h}", bufs=1,
                                    name=f"p1h{h}_{n}_{j}")
                          for h in range(2)]
                    p3 = [psum.tile([128, CHUNK], F32, tag=f"e3h{h}", bufs=2,
                                    name=f"p3h{h}_{n}_{j}")
                          for h in range(2)]
                    e_state[(n, j)] = (p1, p3)
                    for h in range(2):
                        nc.tensor.matmul(
                            p1[h][:],
                            w1_t[64 * h : 64 * h + 64, :],
                            ss[64 * h : 64 * h + 64,
                               1 + y0 : 1 + y0 + ROWS_PER_CHUNK, 1 : 1 + W],
                            start=True,
                            stop=True,
                        )
                p1, p3 = e_state[(n, j)]
                for t in taps:
                    dy, dx = t // 3, t % 3
                    for h in range(2):
                        nc.tensor.matmul(
                            p3[h][:],
                            w3_t[64 * h : 64 * h + 64, t, :],
                            ss[64 * h : 64 * h + 64,
                               y0 + dy : y0 + dy + ROWS_PER_CHUNK,
                               dx : dx + W],
                            start=(t == 0),
                            stop=(t == 8),
                        )
            def expand_chunk_evict(n, j):
                p1, p3 = e_state.pop((n, j))
                last = n == N_IMG - 1 and j == N_CHUNK - 1
                g, half = j // 2, j % 2
                gw = 1 if j == N_CHUNK - 1 else 2
                if half == 0:
                    for role in range(4):
                        out_stage[role] = opool.tile(
                            [128, gw, CHUNK], odt, tag=f"o{role}",
                            name=f"o{role}_{n}_{g}")
                for h in range(2):
                    nc.vector.tensor_scalar(
                        out_stage[h][:, half, :], p1[h][:],
                        b1_t[:, h : h + 1], 0.0,
                        op0=mybir.AluOpType.add, op1=mybir.AluOpType.max,
                    )
                nc.scalar.activation(out_stage[2][:, half, :],
                                     p3[0][:], RELU, bias=b3_t[:, 0:1])
                if last:
                    # final chunk only: second e3 half evicts on DVE so the
                    # two evictions run concurrently instead of queuing on
                    # ACT — shortens the drain, touches nothing early
                    nc.vector.tensor_scalar(
                        out_stage[3][:, half, :], p3[1][:],
                        b3_t[:, 1:2], 0.0,
                        op0=mybir.AluOpType.add, op1=mybir.AluOpType.max,
                    )
                else:
                    nc.scalar.activation(out_stage[3][:, half, :],
                                         p3[1][:], RELU, bias=b3_t[:, 1:2])
                if half + 1 == gw:
                    # final group only: fan the 4 drain DMAs across three
                    # queues instead of serializing ~2us on sync
                    engines = [nc.sync] * 4
                    if last:
                        engines = [nc.sync, nc.sync, nc.scalar, nc.gpsimd]
                    for role in range(4):
                        ch0 = 128 * role
                        engines[role].dma_start(
                            out_d[n, ch0 : ch0 + 128,
                                  2 * g * CHUNK : (2 * g + gw) * CHUNK],
                            out_stage[role][:],
                        )

            PREFETCH = 4
            chunks = [(n, j) for n in range(N_IMG) for j in range(N_CHUNK)]
            groups = []
            for n in range(N_IMG):
                for g in range((N_CHUNK + 1) // 2):
                    groups.append((n, g))
            for gi in range(min(PREFETCH, len(groups))):
                load_group(*groups[gi])
            st = {"next_load": PREFETCH}
            ALL = list(range(9))

            def sq(i):
                n2, j2 = chunks[i]
                if (n2, j2 // 2) not in x_tiles and st["next_load"] < len(groups):
                    load_group(*groups[st["next_load"]])
                    st["next_load"] += 1
                squeeze_chunk(n2, j2)

            # Fill ramp at squeeze-lead 2 with per-chunk interleave — its
            # natural x-wait stalls (~2.5us around 13-17us) are what lets
            # the DVFS governor step the PE to the full 2.4GHz p-state.
            # Steady state then emits squeeze in 2-chunk batches so the PE
            # pays the ~98ns 64<->128-row stationary reconfiguration twice
            # per two chunks instead of four times; once ramped, the
            # governor sustains 2.4GHz through the packed batch stream.
            sq(0)
            sq(1)
            sq(2)
            expand_chunk_mm(*chunks[0], ALL)
            expand_chunk_evict(*chunks[0])
            sq(3)
            expand_chunk_mm(*chunks[1], ALL)
            expand_chunk_evict(*chunks[1])
            for i in range(2, len(chunks), 2):
                if i + 2 < len(chunks):
                    sq(i + 2)
                if i + 3 < len(chunks):
                    sq(i + 3)
                expand_chunk_mm(*chunks[i], ALL)
                expand_chunk_evict(*chunks[i])
                expand_chunk_mm(*chunks[i + 1], ALL)
                expand_chunk_evict(*chunks[i + 1])

    nc.compile()
    return nc


_NC_CACHE = {}


def _get_nc(in_bf16=IN_BF16, exp_bf16=EXP_BF16, out_bf16=OUT_BF16):
    key = (in_bf16, exp_bf16, out_bf16)
    if key not in _NC_CACHE:
        _NC_CACHE[key] = _build(in_bf16, exp_bf16, out_bf16)
    return _NC_CACHE[key]


def _pack_inputs(x, squeeze_w, squeeze_b, expand1x1_w, expand1x1_b,
                 expand3x3_w, expand3x3_b, in_bf16=IN_BF16, exp_bf16=EXP_BF16):
    f = np.float32
    xdt = ml_dtypes.bfloat16 if in_bf16 else f
    edt = ml_dtypes.bfloat16 if exp_bf16 else f
    wsq = np.ascontiguousarray(
        np.tile(squeeze_w, (2, 1))
        .T.reshape(K_TILES, 128, 128)
        .transpose(1, 0, 2)
    ).astype(xdt)
    w1 = np.concatenate(
        [expand1x1_w[:128].T, expand1x1_w[128:].T], axis=0
    ).astype(edt)
    w3e = expand3x3_w.reshape(2, 128, C_SQ, 9)
    w3 = np.ascontiguousarray(w3e.transpose(0, 2, 3, 1)).reshape(128, 9, 128)
    w3 = w3.astype(edt)
    bsq = np.tile(squeeze_b, 2).reshape(128, 1).astype(f)
    b1 = np.ascontiguousarray(expand1x1_b.reshape(2, 128).T).astype(f)
    b3 = np.ascontiguousarray(expand3x3_b.reshape(2, 128).T).astype(f)
    xs = np.ascontiguousarray(
        x.reshape(N_CORES, N_IMG, K_TILES, 128, HW).transpose(0, 1, 3, 2, 4)
    ).astype(xdt)
    return xs, {"wsq": wsq, "w1": w1, "w3": w3, "bsq": bsq, "b1": b1, "b3": b3}


def _run(inputs, trace=False, in_bf16=IN_BF16, exp_bf16=EXP_BF16,
         out_bf16=OUT_BF16):
    from concourse import bass_utils

    nc = _get_nc(in_bf16, exp_bf16, out_bf16)
    xs, weights = _pack_inputs(**inputs, in_bf16=in_bf16, exp_bf16=exp_bf16)
    in_maps = [{"x": xs[c], **weights} for c in range(N_CORES)]
    res = bass_utils.run_bass_kernel_spmd(
        nc, in_maps, core_ids=list(range(N_CORES)), trace=trace
    )
    out = np.concatenate([res.results[c]["out"] for c in range(N_CORES)], axis=0)
    return out.reshape(N_TOTAL, 2 * C_E, H, W).astype(np.float32), res


def kernel(**inputs) -> np.ndarray:
    inputs = {k: np.asarray(v, dtype=np.float32) for k, v in inputs.items()}
    out, _ = _run(inputs, trace=False)
    return out
